# revision 35
# baseline (speedup 1.0000x reference)
"""Two-layer GCN (PyG GCNConv x2 + ReLU) on 8 Trainium2 NeuronCores.

v2 strategy:
  - Nodes packed into 128-slot tiles balanced by in-degree (LPT), 49
    tiles/core. Each core computes g1 = dinv * (x @ W1) for its own nodes
    (f16 GEMM), writes an fp8 message table (f32-packed), and ONE AllGather
    per layer builds the full table in shared DRAM. The f16 self-loop values
    stay SBUF-resident (never leave the core).
  - Per-edge messages are fetched with dma_gather. Gather indices are int16,
    so each tile does two gathers: sources with global row < 32768 ("lo")
    and the rest ("hi", index relative to 32768).
  - Aggregation: one-hot matrices are PRECOMPUTED ON HOST as fp8 (f32-packed)
    and DMA-loaded; aggregation runs as fp8 DoubleRow matmul pairs (two
    128-edge blocks per matmul at half cost). Message buffers are memset once
    so padded blocks read zeros (never NaN).
  - Layer 2: g2 = dinv * (a1 @ W2) per tile (f16), messages stored as a
    hi/lo fp8 split (hi = fp8(g2), lo = fp8(g2 - hi)); aggregation does two
    DoubleRow passes (hi and lo) which halves PE cost vs bf16 with BETTER
    accuracy. Self-loop values f16, SBUF-resident.
  - Biases from setup_inputs() are always zero; a nonzero bias falls back to
    a diag(sqrt deg) @ bias_rep matmul path (kept for generality).
  - DMA issue is spread over SP and Act queues; fp8 conversions run on DVE.
"""

import math
import heapq

import numpy as np
import ml_dtypes

from concourse import bacc, mybir
from concourse.tile import TileContext
from concourse.bass_utils import run_bass_kernel_spmd

F16 = np.float16
F8 = ml_dtypes.float8_e4m3fn
N_CORES = 8
SPLIT = 32768  # int16 gather index limit

# cost-model predicted makespan (ns) of the last _build_nc, for diagnostics
LAST_PREDICTED_NS = None


def _capture_schedule(tc_cls):
    orig = tc_cls.schedule_and_allocate

    def patched(self, validate_deps=False):
        global LAST_PREDICTED_NS
        r = orig(self, validate_deps)
        try:
            LAST_PREDICTED_NS = int(r[1].time)
        except Exception:
            pass
        return r

    if getattr(tc_cls, "_gnn_patched", False):
        return
    tc_cls.schedule_and_allocate = patched
    tc_cls._gnn_patched = True


_capture_schedule(TileContext)

_CFG = dict(
    N=50000,
    E=800000,
    IN=768,
    HID=512,
    OUT=256,
    T=49,  # tiles per core
)


def _pack_idx(idx_linear):
    """[n16] int (n16 % 16 == 0) -> [128, n16//16] int16 in dma_gather layout."""
    num = idx_linear.shape[0]
    a = idx_linear.reshape(num // 16, 16).T.astype(np.int16)
    return np.tile(a, (8, 1))


def _build_nc(cfg, meta):
    IN, HID, OUT = cfg["IN"], cfg["HID"], cfg["OUT"]
    T = cfg["T"]
    PC = T * 128
    NPAD = PC * N_CORES
    NK1 = IN // 128
    NK2 = HID // 128
    KL, KH = meta["KL"], meta["KH"]      # per-tile lo/hi block counts
    NBE = meta["NBE"]                    # per-tile even total blocks (matmul)
    NBMAX_E = meta["NBMAX_E"]
    CMAX = meta["CMAX"]                  # idx cols per tile (uniform stride)
    has_bias = meta["has_bias"]

    f32 = mybir.dt.float32
    f16 = mybir.dt.float16
    f8 = mybir.dt.float8e4
    i16 = mybir.dt.int16

    PK1 = HID // 4   # f32-packed width, layer-1 fp8 table (512B rows)
    PK2 = OUT // 2   # f32-packed width, layer-2 hi|lo fp8 table (512B rows)
    LO_ROWS = SPLIT

    nc = bacc.Bacc(None, target_bir_lowering=False, debug=False)
    xT_p = nc.declare_dram_parameter("xT", [128, IN // 128, PC], f16, isOutput=False)
    w1_p = nc.declare_dram_parameter("w1p", [128, NK1 * HID], f16, isOutput=False)
    w2_p = nc.declare_dram_parameter("w2p", [128, NK2 * OUT], f16, isOutput=False)
    ident_p = nc.declare_dram_parameter("identh", [128, 128], f16, isOutput=False)
    dinv_p = nc.declare_dram_parameter("dinvT", [128, T], f32, isOutput=False)
    idx_p = nc.declare_dram_parameter("idxt", [128, T * CMAX], i16, isOutput=False)
    oh_p = nc.declare_dram_parameter("oht", [T * 128, NBMAX_E * 32], f32, isOutput=False)
    if has_bias:
        b1_p = nc.declare_dram_parameter("b1r", [128, HID], f16, isOutput=False)
        b2_p = nc.declare_dram_parameter("b2r", [128, OUT], f16, isOutput=False)
        dsq_p = nc.declare_dram_parameter("dsqT", [128, T], f32, isOutput=False)
    out_p = nc.declare_dram_parameter("out", [PC, OUT], f16, isOutput=True)

    with TileContext(nc) as tc:
        with (
            tc.tile_pool(name="const", bufs=1) as cpool,
            tc.tile_pool(name="work", bufs=2) as wpool,
            tc.tile_pool(name="psum", bufs=2, space="PSUM") as ppool,
            tc.tile_pool(name="dram", bufs=1, space="DRAM") as dpool,
        ):
            # ---- internal DRAM ----
            g1s = dpool.tile([PC, PK1], f32, name="g1s")
            g2s = dpool.tile([PC, PK2], f32, name="g2s")
            g1f = dpool.tile([NPAD, PK1], f32, name="g1f", addr_space="Shared")
            g2f = dpool.tile([NPAD, PK2], f32, name="g2f", addr_space="Shared")

            # ---- constants / resident tables ----
            w1sb = cpool.tile([128, NK1 * HID], f16, name="w1sb")
            nc.scalar.dma_start(out=w1sb[:, : 2 * HID], in_=w1_p[:, : 2 * HID])
            nc.scalar.dma_start(out=w1sb[:, 2 * HID :], in_=w1_p[:, 2 * HID :])
            dnv = cpool.tile([128, T], f32, name="dnv")
            nc.scalar.dma_start(out=dnv[:, :], in_=dinv_p[:, :])
            w2sb = cpool.tile([128, NK2 * OUT], f16, name="w2sb")
            idn = cpool.tile([128, 128], f16, name="idn")
            if has_bias:
                b1sb = cpool.tile([128, HID], f16, name="b1sb")
                nc.scalar.dma_start(out=b1sb[:, :], in_=b1_p[:, :])
                b2sb = cpool.tile([128, OUT], f16, name="b2sb")
                nc.scalar.dma_start(out=b2sb[:, :], in_=b2_p[:, :])
                dsq = cpool.tile([128, T], f32, name="dsq")
                nc.scalar.dma_start(out=dsq[:, :], in_=dsq_p[:, :])

            g1self = cpool.tile([128, T, HID], f16, name="g1self")   # 49KB/part
            g2self = cpool.tile([128, T, OUT], f16, name="g2self")   # 24.5KB/part
            ixall = cpool.tile([128, T, CMAX], i16, name="ixall")    # 13.3KB/part
            IXCH = 8
            for t0 in range(0, T, IXCH):
                tn = min(IXCH, T - t0)
                nc.gpsimd.dma_start(
                    out=ixall[:, t0 : t0 + tn, :],
                    in_=idx_p[:, t0 * CMAX : (t0 + tn) * CMAX],
                )

            # message buffers (fp8 f32-packed), memset once so padded blocks
            # and gather tails read zeros (never NaN)
            MSG_BUFS = 4
            msgs = []
            for i in range(MSG_BUFS):
                mt = cpool.tile([128, NBMAX_E, 128], f32, name=f"msg{i}")
                nc.gpsimd.memset(mt[:, :, :], 0.0)
                msgs.append(mt)

            diags = []
            if has_bias:
                for t in range(T):
                    diag = cpool.tile([128, 128], f16, name=f"dg{t}")
                    nc.vector.tensor_scalar(
                        diag[:, :], idn[:, :], dsq[:, t : t + 1], None,
                        mybir.AluOpType.mult,
                    )
                    diags.append(diag)

            # ---- phase 1: g1 = dinv * (x @ W1) for own nodes ----
            CH = 5
            for ch0 in range(0, T, CH):
                chn = min(CH, T - ch0)
                xk = wpool.tile([128, NK1, CH * 128], f16, tag="xk", bufs=2)
                nc.sync.dma_start(
                    out=xk[:, : NK1 // 2, : chn * 128],
                    in_=xT_p[:, : NK1 // 2, ch0 * 128 : (ch0 + chn) * 128],
                )
                nc.sync.dma_start(
                    out=xk[:, NK1 // 2 :, : chn * 128],
                    in_=xT_p[:, NK1 // 2 :, ch0 * 128 : (ch0 + chn) * 128],
                )
                for sub in range(chn):
                    t = ch0 + sub
                    ps = ppool.tile([128, HID], f32, tag="p1", bufs=4)
                    for k in range(NK1):
                        nc.tensor.matmul(
                            ps[:, :],
                            xk[:, k, sub * 128 : (sub + 1) * 128],
                            w1sb[:, k * HID : (k + 1) * HID],
                            start=(k == 0),
                            stop=(k == NK1 - 1),
                        )
                    ge8 = wpool.tile([128, PK1], f32, tag="ge8", bufs=3)
                    nc.vector.tensor_scalar(
                        ge8[:, :].bitcast(f8), ps[:, :], dnv[:, t : t + 1], None,
                        mybir.AluOpType.mult,
                    )
                    if t >= T - 2:
                        # tail tiles: table write queued ahead of the self copy
                        # so the AllGather dependency clears sooner
                        nc.scalar.dma_start(
                            out=g1s[t * 128 : (t + 1) * 128, :], in_=ge8[:, :]
                        )
                        nc.scalar.activation(
                            g1self[:, t, :], ps[:, :],
                            mybir.ActivationFunctionType.Copy,
                            scale=dnv[:, t : t + 1],
                        )
                    else:
                        nc.scalar.activation(
                            g1self[:, t, :], ps[:, :],
                            mybir.ActivationFunctionType.Copy,
                            scale=dnv[:, t : t + 1],
                        )
                        nc.scalar.dma_start(
                            out=g1s[t * 128 : (t + 1) * 128, :], in_=ge8[:, :]
                        )
            nc.scalar.dma_start(out=w2sb[:, :], in_=w2_p[:, :])
            nc.scalar.dma_start(out=idn[:, :], in_=ident_p[:, :])
            nc.gpsimd.collective_compute(
                "AllGather",
                mybir.AluOpType.bypass,
                ins=[g1s[:, :].opt()],
                outs=[g1f[:, :].opt()],
                replica_groups=[list(range(N_CORES))],
            )

            def load_oh(t, eng):
                oh = wpool.tile([128, NBMAX_E, 32], f32, tag="oh", bufs=3)
                eng.dma_start(
                    out=oh[:, : NBE[t], :],
                    in_=oh_p[t * 128 : (t + 1) * 128, : NBE[t] * 32],
                )
                return oh

            def gather_tile(t, msg, table, pk):
                kl, kh = KL[t], KH[t]
                if kl:
                    nc.gpsimd.dma_gather(
                        msg[:, :kl, :], table[0:LO_ROWS, :], ixall[:, t, : kl * 8],
                        kl * 128, kl * 128, pk, single_packet=False,
                    )
                if kh:
                    nc.gpsimd.dma_gather(
                        msg[:, kl : kl + kh, :], table[LO_ROWS:NPAD, :],
                        ixall[:, t, kl * 8 : (kl + kh) * 8],
                        kh * 128, kh * 128, pk, single_packet=False,
                    )

            def agg_tile(t, msg, oh, width, self_ap, bias_sb, psum_tag, hilo):
                """Aggregate tile t from gathered msgs; returns PSUM tile."""
                ps = ppool.tile([128, width], f32, tag=psum_tag, bufs=4)
                if has_bias:
                    nc.tensor.matmul(ps[:, :], diags[t][:, :], bias_sb[:, :],
                                     start=True, stop=False)
                    nc.tensor.matmul(ps[:, :], idn[:, :], self_ap,
                                     start=False, stop=False)
                else:
                    nc.tensor.matmul(ps[:, :], idn[:, :], self_ap,
                                     start=True, stop=False)
                npair = NBE[t] // 2
                for p in range(npair):
                    lhsT = oh[:, 2 * p : 2 * p + 2, :].bitcast(f8)
                    if hilo:
                        nc.tensor.matmul(
                            ps[:, :], lhsT,
                            msg[:, 2 * p : 2 * p + 2, 0:64].bitcast(f8),
                            start=False, stop=False,
                            perf_mode=mybir.MatmulPerfMode.DoubleRow,
                        )
                        nc.tensor.matmul(
                            ps[:, :], lhsT,
                            msg[:, 2 * p : 2 * p + 2, 64:128].bitcast(f8),
                            start=False, stop=(p == npair - 1),
                            perf_mode=mybir.MatmulPerfMode.DoubleRow,
                        )
                    else:
                        nc.tensor.matmul(
                            ps[:, :], lhsT,
                            msg[:, 2 * p : 2 * p + 2, :].bitcast(f8),
                            start=False, stop=(p == npair - 1),
                            perf_mode=mybir.MatmulPerfMode.DoubleRow,
                        )
                return ps

            # ---- phase 2: layer-1 aggregation + fused layer-2 GEMM ----
            # largest tiles first: the final gather + pipeline tail (which
            # gates the second AllGather) belongs to the smallest tile
            order = sorted(range(T), key=lambda u: -NBE[u])
            for ti, t in enumerate(order):
                oh = load_oh(t, nc.scalar)
                m1 = msgs[ti % MSG_BUFS]
                gather_tile(t, m1, g1f, PK1)
                ps = agg_tile(t, m1, oh, HID, g1self[:, t, :],
                              b1sb if has_bias else None, "p1", hilo=False)

                a1 = wpool.tile([128, HID], f16, tag="a1", bufs=3)
                if ti >= T - 1:
                    nc.scalar.activation(
                        a1[:, :], ps[:, :], mybir.ActivationFunctionType.Relu,
                        scale=dnv[:, t : t + 1],
                    )
                else:
                    nc.vector.tensor_scalar(
                        a1[:, :], ps[:, :], dnv[:, t : t + 1], 0.0,
                        mybir.AluOpType.mult, mybir.AluOpType.max,
                    )

                aT = wpool.tile([128, NK2, 128], f16, tag="aT", bufs=3)
                for k in range(NK2):
                    nc.sync.dma_start_transpose(
                        aT[:, k, :], a1[:, k * 128 : (k + 1) * 128]
                    )
                ps2 = ppool.tile([128, OUT], f32, tag="p2", bufs=4)
                for k in range(NK2):
                    nc.tensor.matmul(
                        ps2[:, :], aT[:, k, :], w2sb[:, k * OUT : (k + 1) * OUT],
                        start=(k == 0), stop=(k == NK2 - 1),
                    )
                nc.scalar.activation(
                    g2self[:, t, :], ps2[:, :], mybir.ActivationFunctionType.Copy,
                    scale=dnv[:, t : t + 1],
                )
                g2m = wpool.tile([128, 2, OUT // 4], f32, tag="g2m", bufs=3)
                nc.vector.tensor_scalar(
                    g2m[:, 0, :].bitcast(f8), ps2[:, :], dnv[:, t : t + 1], None,
                    mybir.AluOpType.mult,
                )
                nc.vector.tensor_tensor(
                    g2m[:, 1, :].bitcast(f8), g2self[:, t, :],
                    g2m[:, 0, :].bitcast(f8), mybir.AluOpType.subtract,
                )
                nc.sync.dma_start(out=g2s[t * 128 : (t + 1) * 128, :], in_=g2m[:, :, :])
            nc.gpsimd.collective_compute(
                "AllGather",
                mybir.AluOpType.bypass,
                ins=[g2s[:, :].opt()],
                outs=[g2f[:, :].opt()],
                replica_groups=[list(range(N_CORES))],
            )

            # ---- phase 3: layer-2 aggregation -> output ----
            for ti, t in enumerate(order):
                oh2 = load_oh(t, nc.sync)
                m2 = msgs[ti % MSG_BUFS]
                gather_tile(t, m2, g2f, PK2)
                ps3 = agg_tile(t, m2, oh2, OUT, g2self[:, t, :],
                               b2sb if has_bias else None, "p2", hilo=True)
                of = wpool.tile([128, OUT], f16, tag="of", bufs=3)
                nc.scalar.activation(
                    of[:, :], ps3[:, :], mybir.ActivationFunctionType.Copy,
                    scale=dnv[:, t : t + 1],
                )
                nc.scalar.dma_start(out=out_p[t * 128 : (t + 1) * 128, :], in_=of[:, :])

    nc.compile()
    return nc


def _preprocess(x, edge_index, W1, b1, W2, b2, cfg):
    N, E = cfg["N"], cfg["E"]
    IN, HID, OUT = cfg["IN"], cfg["HID"], cfg["OUT"]
    T = cfg["T"]
    PC = T * 128
    NPAD = PC * N_CORES
    TT = T * N_CORES

    src = np.asarray(edge_index[0], dtype=np.int64)
    dst = np.asarray(edge_index[1], dtype=np.int64)

    indeg = np.bincount(dst, minlength=N)
    deg = indeg.astype(np.float32) + 1.0
    dinv = 1.0 / np.sqrt(deg)

    # ---- balanced node -> (tile, slot) assignment (LPT greedy) ----
    order = np.argsort(-indeg, kind="stable")
    heap = [(0, t, 0) for t in range(TT)]  # (load, bin, used)
    heapq.heapify(heap)
    row_of_node = np.empty(N, dtype=np.int64)
    for n in order:
        load, t, used = heapq.heappop(heap)
        row_of_node[n] = t * 128 + used
        used += 1
        if used < 128 and t * 128 + used < NPAD:
            heapq.heappush(heap, (load + int(indeg[n]), t, used))
    # bins are (core, tile) pairs laid out core-major: bin b -> core b//T, tile b%T
    # global row of node = bin*128 + slot; identity val mapping for AG tables

    # ---- per-edge quantities ----
    srow = row_of_node[src]
    drow = row_of_node[dst]
    e_core = drow // PC
    e_tile = (drow % PC) // 128
    e_slot = drow % 128
    e_hi = srow >= SPLIT
    e_val = np.where(e_hi, srow - SPLIT, srow)

    # ---- segment counts per (core, tile, half) ----
    segkey = (e_core * T + e_tile) * 2 + e_hi.astype(np.int64)
    cnt = np.bincount(segkey, minlength=TT * 2).reshape(N_CORES, T, 2)

    KL = [math.ceil(int(cnt[:, t, 0].max()) / 128) for t in range(T)]
    KH = [math.ceil(int(cnt[:, t, 1].max()) / 128) for t in range(T)]
    NB = [kl + kh for kl, kh in zip(KL, KH)]
    NBE = [nb + (nb % 2) for nb in NB]  # even for clean DoubleRow pairing
    NBMAX_E = max(NBE)
    CMAX = max(NB) * 8  # idx i16 cols per tile row ((KL+KH)*128/16)

    # ---- per-core tables: idx (gather), one-hot (aggregation) ----
    ordk = np.lexsort((e_val, segkey))
    sv_s = e_val[ordk]
    ss_s = e_slot[ordk]
    seg_start = np.searchsorted(segkey[ordk], np.arange(TT * 2))
    seg_end = np.searchsorted(segkey[ordk], np.arange(TT * 2) + 1)

    idxt = np.zeros((N_CORES, 128, T * CMAX), dtype=np.int16)
    oht = np.zeros((N_CORES, T * 128, NBMAX_E * 32), dtype=np.float32)
    oh_bytes = oht.view(F8).reshape(N_CORES, T * 128, NBMAX_E, 128)
    for c in range(N_CORES):
        for t in range(T):
            kl, kh = KL[t], KH[t]
            col = 0
            for h, nblk in ((0, kl), (1, kh)):
                k = (c * T + t) * 2 + h
                a, b = seg_start[k], seg_end[k]
                n = b - a
                cap = nblk * 128
                assert n <= cap
                seg_idx = np.zeros(cap, dtype=np.int64)
                seg_idx[:n] = sv_s[a:b]
                idxt[c, :, t * CMAX + col : t * CMAX + col + cap // 16] = _pack_idx(seg_idx)
                col += cap // 16
                # one-hot blocks for this half: edge j (block j//128, row j%128)
                bb = 0 if h == 0 else kl
                slots = ss_s[a:b]
                j = np.arange(n)
                oh_bytes[c, t * 128 + (j % 128), bb + j // 128, slots] = np.float32(
                    1.0
                ).astype(F8)

    # ---- dense host tensors ----
    xPermT = np.zeros((IN, NPAD), dtype=np.float32)
    xPermT[:, row_of_node] = np.asarray(x, dtype=np.float32).T
    dinv_row = np.zeros(NPAD, dtype=np.float32)
    dinv_row[row_of_node] = dinv

    NK1, NK2 = IN // 128, HID // 128
    w1p = (
        np.asarray(W1, np.float32).reshape(NK1, 128, HID).transpose(1, 0, 2)
        .reshape(128, NK1 * HID).astype(F16)
    )
    w2p = (
        np.asarray(W2, np.float32).reshape(NK2, 128, OUT).transpose(1, 0, 2)
        .reshape(128, NK2 * OUT).astype(F16)
    )
    identh = np.eye(128, dtype=np.float32).astype(F16)

    b1 = np.asarray(b1, np.float32)
    b2 = np.asarray(b2, np.float32)
    has_bias = bool(np.any(b1) or np.any(b2))

    in_maps = []
    for c in range(N_CORES):
        sl_c = slice(c * PC, (c + 1) * PC)
        m = {
            "xT": xPermT[:, sl_c].reshape(NK1, 128, PC).transpose(1, 0, 2).astype(F16).copy(),
            "w1p": w1p,
            "w2p": w2p,
            "identh": identh,
            "dinvT": dinv_row[sl_c].reshape(T, 128).T.astype(np.float32).copy(),
            "idxt": idxt[c],
            "oht": oht[c],
        }
        if has_bias:
            dsq_row = np.zeros(NPAD, dtype=np.float32)
            dsq_row[row_of_node] = np.sqrt(deg)
            m["b1r"] = np.tile(b1[None, :], (128, 1)).astype(F16)
            m["b2r"] = np.tile(b2[None, :], (128, 1)).astype(F16)
            m["dsqT"] = dsq_row[sl_c].reshape(T, 128).T.astype(np.float32).copy()
        in_maps.append(m)

    meta = dict(
        KL=KL, KH=KH, NBE=NBE, NBMAX_E=NBMAX_E, CMAX=CMAX,
        has_bias=has_bias,
        row_of_node=row_of_node,
    )
    return in_maps, meta


def kernel(x, edge_index, W1, b1, W2, b2):
    cfg = _CFG
    N, OUT = cfg["N"], cfg["OUT"]
    PC = cfg["T"] * 128
    in_maps, meta = _preprocess(x, edge_index, W1, b1, W2, b2, cfg)
    nc = _build_nc(cfg, meta)
    import os
    if os.environ.get("GNN_SIM"):
        from concourse import bass_interp

        sim = bass_interp.MultiCoreSim(nc, N_CORES)
        for c in range(N_CORES):
            for k, v in in_maps[c].items():
                sim.cores[c].tensor(k)[:] = v
        sim.simulate()
        results = [
            {"out": np.array(sim.cores[c].tensor("out"))} for c in range(N_CORES)
        ]
    else:
        res = run_bass_kernel_spmd(nc, in_maps, core_ids=list(range(N_CORES)))
        results = res.results
    out = np.empty((N, OUT), dtype=np.float32)
    row = meta["row_of_node"]
    core = row // PC
    local = row % PC
    for c in range(N_CORES):
        m = core == c
        out[np.where(m)[0]] = results[c]["out"][local[m]].astype(np.float32)
    return out
